# revision 7
# baseline (speedup 1.0000x reference)
"""GPT2 attention (B=2, S=2048, E=1024, H=16) on 8 NeuronCores.

Sharding: tensor-parallel over heads — 2 heads per core. Each core computes
qkv^T for its heads, causal attention in transposed-score layout (k on
partitions, q on free dim), then a partial output projection over its 128
ctx dims. Host sums the 8 partials and adds b_proj.

Compute is bf16 (f32 PSUM accumulation); validated rel-l2 ~4e-3 vs the f32
reference. Causal structure: only lower-triangular 128x512 score blocks are
computed; diagonal blocks are masked via gpsimd affine_select after exp.
Softmax denominator comes free from a ones-column appended to V (PV matmul
row 64); normalization is a rank-1 reciprocal broadcast matmul + DVE mul.
"""
import os
import numpy as np
import ml_dtypes

import concourse.bass as bass
import concourse.bacc as bacc
import concourse.tile as tile
from concourse import mybir
from concourse import masks
from concourse.bass_utils import run_bass_kernel_spmd

BF16 = ml_dtypes.bfloat16
B, S, E, H, D = 2, 2048, 1024, 16, 64
T = B * S                 # 4096 tokens
NCORE = 8
HPC = H // NCORE          # 2 heads per core
NEG = -10000.0
SCALE = D ** -0.5
F32 = mybir.dt.float32
BF = mybir.dt.bfloat16
EXP = mybir.ActivationFunctionType.Exp

_built = {}


def _build():
    if "nc" in _built:
        return _built["nc"]
    nc = bacc.Bacc()
    hsT = nc.declare_dram_parameter("hsT", [E, T], BF, isOutput=False)
    wqkv = nc.declare_dram_parameter("wqkv", [E, 3 * HPC * D], BF, isOutput=False)
    bqkv = nc.declare_dram_parameter("bqkv", [1, 3 * HPC * D], BF, isOutput=False)
    wpT = nc.declare_dram_parameter("wpT", [HPC * D, E], BF, isOutput=False)
    padneg = nc.declare_dram_parameter("padneg", [128, 32], F32, isOutput=False)
    out = nc.declare_dram_parameter("out", [T, E], BF, isOutput=True)

    NQ = S // 512             # 4 q-tiles of 512 per batch
    NK = S // 128             # 16 k-chunks of 128 per batch

    with tile.TileContext(nc) as tc:
        with (
            tc.tile_pool(name="const", bufs=1) as constp,
            tc.tile_pool(name="hst", bufs=4) as hstp,
            tc.tile_pool(name="big", bufs=1) as bigp,
            tc.tile_pool(name="expt", bufs=2) as exptp,
            tc.tile_pool(name="small", bufs=3) as smallp,
            tc.tile_pool(name="outp", bufs=4) as outp,
            tc.tile_pool(name="ps_qkv", bufs=4, space="PSUM") as ps_qkv,
            tc.tile_pool(name="ps_sc", bufs=2, space="PSUM") as ps_sc,
            tc.tile_pool(name="ps_ctx", bufs=2, space="PSUM") as ps_ctx,
        ):
            # ---- constants ----
            wqkv_sb = constp.tile([128, 8, 384], BF)
            nc.sync.dma_start(
                out=wqkv_sb, in_=wqkv.rearrange("(kc p) m -> p kc m", p=128)
            )
            bq_sb = constp.tile([1, 384], BF)
            nc.sync.dma_start(out=bq_sb, in_=bqkv[:])
            wpT_sb = constp.tile([128, E], BF)
            nc.sync.dma_start(out=wpT_sb, in_=wpT[:])
            pad_sb = constp.tile([128, 32], F32)
            nc.sync.dma_start(out=pad_sb, in_=padneg[:])
            ident = constp.tile([128, 128], BF)
            masks.make_identity(nc, ident[:])
            ones_bf = constp.tile([1, 512], BF)
            nc.vector.memset(ones_bf, 1.0)
            ones64 = constp.tile([1, 64], F32)
            nc.vector.memset(ones64, 1.0)

            qT = bigp.tile([128, T], BF)       # rows: h0 dims 0-63, h1 dims 64-127
            kT = bigp.tile([128, T], BF)
            ctxT = bigp.tile([128, T], BF)
            # v in natural layout: per 128-token chunk tt, 130 cols:
            # [0:64]=h0 dims, [64]=ones, [65:129]=h1 dims, [129]=ones
            vnat = bigp.tile([128, 32, 130], BF)
            nc.vector.memset(vnat[:, :, 64:65], 1.0)
            nc.vector.memset(vnat[:, :, 129:130], 1.0)

            hsT_r = hsT.rearrange("(kc p) t -> kc p t", p=128)  # [8,128,4096]

            # ---- phase B: qkv^T = Wc @ hsT + b, and v transpose ----
            for n in range(8):                  # 512-token tiles (batch0 first)
                pm = [ps_qkv.tile([128, 512], F32, tag="qkv", name=f"qkv{n}_{m}")
                      for m in range(3)]
                for m in range(3):
                    nc.tensor.matmul(
                        pm[m], lhsT=bq_sb[:, m * 128:(m + 1) * 128], rhs=ones_bf,
                        start=True, stop=False,
                    )
                for k in range(8):
                    ht = hstp.tile([128, 512], BF, tag="ht")
                    nc.gpsimd.dma_start(out=ht, in_=hsT_r[k, :, n * 512:(n + 1) * 512])
                    for m in range(3):
                        nc.tensor.matmul(
                            pm[m], lhsT=wqkv_sb[:, k, m * 128:(m + 1) * 128], rhs=ht,
                            start=False, stop=(k == 7),
                        )
                nc.vector.tensor_copy(qT[:, n * 512:(n + 1) * 512], pm[0])
                nc.vector.tensor_copy(kT[:, n * 512:(n + 1) * 512], pm[1])
                vtmp = smallp.tile([128, 512], BF, tag="vtmp")
                nc.vector.tensor_copy(vtmp, pm[2])
                for t in range(4):
                    tt = n * 4 + t
                    pst = ps_ctx.tile([128, 128], BF, tag="ctx")
                    nc.tensor.transpose(pst[:], vtmp[:, t * 128:(t + 1) * 128], ident[:])
                    nc.vector.tensor_copy(vnat[:, tt, 0:64], pst[:, 0:64])
                    nc.vector.tensor_copy(vnat[:, tt, 65:129], pst[:, 64:128])

            # ---- phase C: causal attention, transposed scores ----
            for b in range(B):
                for h in range(HPC):
                    hs_, he_ = h * 64, (h + 1) * 64
                    for qj in range(NQ):
                        q0 = b * S + qj * 512
                        nk = 4 * qj + 4        # causal: k-chunks 0..4qj+3
                        expt = exptp.tile([128, 16, 512], BF, tag="expt")
                        ctxp = ps_ctx.tile([128, 512], F32, tag="ctx")
                        for ki in range(nk):
                            scp = ps_sc.tile([128, 512], F32, tag="sc")
                            nc.tensor.matmul(
                                scp,
                                lhsT=kT[hs_:he_, b * S + ki * 128: b * S + (ki + 1) * 128],
                                rhs=qT[hs_:he_, q0: q0 + 512],
                                start=True, stop=True,
                            )
                            nc.scalar.activation(
                                out=expt[:, ki, :], in_=scp, func=EXP,
                                bias=pad_sb[:, b * 16 + ki: b * 16 + ki + 1],
                                scale=SCALE,
                            )
                            d = ki - 4 * qj
                            if d >= 0:  # diagonal block: zero where k > q
                                nc.gpsimd.affine_select(
                                    out=expt[:, ki, :], in_=expt[:, ki, :],
                                    compare_op=mybir.AluOpType.is_ge, fill=0.0,
                                    base=-(128 * d), channel_multiplier=-1,
                                    pattern=[[1, 512]],
                                )
                            nc.tensor.matmul(
                                ctxp[0:65, :],
                                lhsT=vnat[:, b * 16 + ki, h * 65:(h + 1) * 65],
                                rhs=expt[:, ki, :],
                                start=(ki == 0), stop=(ki == nk - 1),
                            )
                        # normalize by the denominator in row 64
                        rec = smallp.tile([1, 512], F32, tag="rec")
                        nc.vector.reciprocal(rec, ctxp[64:65, :])
                        bcp = ps_sc.tile([128, 512], F32, tag="sc")
                        nc.tensor.matmul(bcp[0:64, :], lhsT=ones64, rhs=rec,
                                         start=True, stop=True)
                        bcs = smallp.tile([64, 512], F32, tag="bcs")
                        nc.vector.tensor_copy(bcs, bcp[0:64, :])
                        nc.vector.tensor_mul(
                            ctxT[hs_:he_, q0: q0 + 512], ctxp[0:64, :], bcs,
                        )

            # ---- phase D: partial out projection ----
            for mt in range(32):
                for n2 in range(2):
                    pp = ps_qkv.tile([128, 512], F32, tag="qkv")
                    nc.tensor.matmul(
                        pp, lhsT=ctxT[:, mt * 128:(mt + 1) * 128],
                        rhs=wpT_sb[:, n2 * 512:(n2 + 1) * 512],
                        start=True, stop=True,
                    )
                    ot = outp.tile([128, 512], BF, tag="ot")
                    nc.vector.tensor_copy(ot, pp)
                    nc.gpsimd.dma_start(
                        out=out[mt * 128:(mt + 1) * 128, n2 * 512:(n2 + 1) * 512],
                        in_=ot,
                    )
    nc.finalize()
    _built["nc"] = nc
    return nc


def kernel(hidden_states, attention_mask, W_attn, b_attn, W_proj, b_proj,
           _trace=False):
    hs = np.asarray(hidden_states, np.float32).reshape(T, E)
    hsT = np.ascontiguousarray(hs.T).astype(BF16)
    mask = np.asarray(attention_mask)
    padfull = np.where(mask != 0, 0.0, NEG).astype(np.float32)      # [B,S]
    pad = np.ascontiguousarray(
        padfull.reshape(B * 16, 128).T                               # [128, 32]
    )
    W_attn = np.asarray(W_attn, np.float32)
    W_proj = np.asarray(W_proj, np.float32)
    b_attn = np.asarray(b_attn, np.float32)

    in_maps = []
    for c in range(NCORE):
        rows = np.concatenate(
            [np.arange(sec * E + c * 128, sec * E + (c + 1) * 128)
             for sec in range(3)]
        )
        wq = np.ascontiguousarray(W_attn[rows].T).astype(BF16)       # [1024,384]
        bq = np.ascontiguousarray(b_attn[rows][None, :]).astype(BF16)
        wp = np.ascontiguousarray(W_proj[:, c * 128:(c + 1) * 128].T).astype(BF16)
        in_maps.append(
            {"hsT": hsT, "wqkv": wq, "bqkv": bq, "wpT": wp, "padneg": pad}
        )

    nc = _build()
    res = run_bass_kernel_spmd(nc, in_maps, list(range(NCORE)), trace=_trace)
    parts = np.stack([np.asarray(r["out"], np.float32) for r in res.results])
    outv = parts.sum(axis=0) + np.asarray(b_proj, np.float32)[None, :]
    out = outv.reshape(B, S, E).astype(np.float32)
    if _trace:
        return out, res
    return out
